# revision 30
# baseline (speedup 1.0000x reference)
"""Batch-assign-probability (VQ codebook softmax) kernel for 8 Trainium2 cores.

Math: for each valid row x (D=512), over K=256 centers c_k:
    softmax_k(-||x - c_k||^2) == softmax_k(2 x.c_k - ||c_k||^2)
(the ||x||^2 term is constant over k and cancels in softmax).

Sharding: batch B=16 split across 8 cores (2 batches = 2048 valid rows per
core); the small centers table is replicated. Host prep: slice the valid
(unmasked) timesteps, transpose x to [D, rows] so the contraction dim lands
on SBUF partitions, fold the 2x scale into ct = (2*centers)^T, and split
x / ct into bf16 hi+lo pairs for a 3-pass full-rate matmul:
    x.ct ~= xh.cth + xh.ctl + xl.cth        (error ~2e-4 relative)
The -||c||^2 bias is folded in as one contraction-dim-3 matmul against a
3-level bf16 split of the bias (ones rows on the x side). Host packs each
DMA's source region fully contiguous (8KB-per-partition runs).

Device (per core, Tile framework):
  - load ct hi/lo + bias once; stream x hi/lo in row-groups (small first
    group so the PE starts early, small last group so the exposed softmax
    tail is short); per 128-row tile: 12 bf16 matmuls + 1 bias matmul ->
    PSUM logits [128,256]; reduce_max(negate) -> ACT exp(bias=-max,
    accum sum) -> reciprocal -> scale -> group out DMA.
"""

import numpy as np
import ml_dtypes

import concourse.bacc as bacc
import concourse.tile as tile
from concourse import mybir
from concourse.bass_utils import run_bass_kernel_spmd

B, T, W, C, K = 16, 2048, 512, 1, 256
VALID_T = 1024
D = W * C                       # 512
N_CORES = 8
B_PER_CORE = B // N_CORES       # 2
ROWS = B_PER_CORE * VALID_T     # 2048 rows per core
P = 128
D_CHUNKS = D // P               # 4
GROUPS = [256, 384, 512, 512, 256, 128]   # rows per x/out DMA group
N_WARM_MM = 10                  # dummy matmuls to lift the PE HAM clock-gate
assert sum(GROUPS) == ROWS
X_TOTAL = P * 2 * D_CHUNKS * ROWS    # flat bf16 element count of x param

BF16_NP = ml_dtypes.bfloat16

_CACHE: dict = {}


def _build_bass():
    f32 = mybir.dt.float32
    bf16 = mybir.dt.bfloat16
    nc = bacc.Bacc()
    # x hi/lo, group-major, fully contiguous per group: for each group g
    # (R rows), block [128p, 2h, 4c, R] flattened.
    xp = nc.declare_dram_parameter("xp", [X_TOTAL], bf16, isOutput=False)
    # ct hi block then lo block, each [128p, 4c, 256k] contiguous.
    ctp = nc.declare_dram_parameter("ctp", [2 * P * D_CHUNKS * K], bf16,
                                    isOutput=False)
    bias3 = nc.declare_dram_parameter("bias3", [3, K], bf16, isOutput=False)
    ones3 = nc.declare_dram_parameter("ones3", [3, P], bf16, isOutput=False)
    out = nc.declare_dram_parameter("out", [ROWS, K], f32, isOutput=True)

    out_v = out.rearrange("(t p) k -> p t k", p=P)       # [128, 16, 256]
    ct_half = P * D_CHUNKS * K

    with tile.TileContext(nc) as tc:
        with (
            tc.tile_pool(name="singles", bufs=1) as singles,
            tc.tile_pool(name="xpool", bufs=1) as xpool,
            tc.tile_pool(name="opool", bufs=3) as opool,
            tc.tile_pool(name="small", bufs=8) as small,
            tc.tile_pool(name="psum", bufs=7, space="PSUM") as psum,
            tc.tile_pool(name="psum_warm", bufs=1, space="PSUM") as psum_warm,
        ):
            # DMA order on the sync HWDGE FIFO: ct_hi -> x group 0 -> ct_lo
            # -> remaining x groups. The first four matmuls only need ct_hi
            # + x0, so the PE starts as early as possible. bias/ones ride
            # the scalar HWDGE ring to stay off the sync FIFO.
            ct_sb = singles.tile([P, 2, D_CHUNKS, K], bf16)
            xgs = []
            xoff = 0

            def x_dma(g, R):
                xg = xpool.tile([P, 2, D_CHUNKS, R], bf16, tag=f"xg{g}")
                n = P * 2 * D_CHUNKS * R
                nc.sync.dma_start(
                    out=xg[:],
                    in_=xp[xoff:xoff + n].rearrange(
                        "(p h c r) -> p h c r", p=P, h=2, c=D_CHUNKS),
                )
                xgs.append(xg)
                return n

            def ct_dma(h):
                nc.sync.dma_start(
                    out=ct_sb[:, h],
                    in_=ctp[h * ct_half:(h + 1) * ct_half].rearrange(
                        "(p c k) -> p c k", p=P, c=D_CHUNKS),
                )

            ct_dma(0)
            ct_dma(1)
            xoff += x_dma(0, GROUPS[0])
            bias_sb = singles.tile([3, K], bf16)
            nc.scalar.dma_start(out=bias_sb[:], in_=bias3[:])
            ones_sb = singles.tile([3, P], bf16)
            nc.scalar.dma_start(out=ones_sb[:], in_=ones3[:])
            for g, R in enumerate(GROUPS[1:], start=1):
                xoff += x_dma(g, R)

            # PE warm-up: dummy matmuls on scratch data keep the PE busy
            # through the HAM activity window while the first x DMA lands,
            # so the real matmul stream runs at 2.4 GHz from the start.
            warm_sb = singles.tile([P, 512], bf16)
            nc.vector.memset(warm_sb[:], 0.0)
            warm_ps = psum_warm.tile([P, 512], f32, tag="warm")
            for _ in range(N_WARM_MM):
                nc.tensor.matmul(
                    warm_ps[:], lhsT=warm_sb[:, :P], rhs=warm_sb[:],
                    start=True, stop=True,
                )

            t0 = 0  # running 128-row tile index
            for g, R in enumerate(GROUPS):
                xg = xgs[g]
                subtiles = R // P
                og = opool.tile([P, subtiles, K], f32, tag="og")
                esum_g = small.tile([P, subtiles], f32, tag="esum")
                # pair subtiles into one full PSUM bank: shared reduce_max,
                # per-group reciprocal + broadcast multiply
                for s0 in range(0, subtiles, 2):
                    pair = min(2, subtiles - s0)
                    ps = psum.tile([P, pair, K], f32, tag="ps")
                    for j in range(pair):
                        s = s0 + j
                        rsl = slice(s * P, (s + 1) * P)
                        first = True
                        for xh_i, ct_i in ((0, 0), (0, 1), (1, 0)):
                            for c in range(D_CHUNKS):
                                nc.tensor.matmul(
                                    ps[:, j, :],
                                    lhsT=xg[:, xh_i, c, rsl],
                                    rhs=ct_sb[:, ct_i, c, :],
                                    start=first,
                                    stop=False,
                                )
                                first = False
                        nc.tensor.matmul(
                            ps[:, j, :],
                            lhsT=ones_sb[:, :],
                            rhs=bias_sb[:, :],
                            start=False,
                            stop=True,
                        )
                    negm = small.tile([P, pair], f32, tag="negm")
                    nc.vector.reduce_max(
                        out=negm[:], in_=ps[:], axis=mybir.AxisListType.X, negate=True
                    )
                    for j in range(pair):
                        nc.scalar.activation(
                            out=og[:, s0 + j, :],
                            in_=ps[:, j, :],
                            func=mybir.ActivationFunctionType.Exp,
                            bias=negm[:, j:j + 1],
                            scale=1.0,
                            accum_out=esum_g[:, s0 + j:s0 + j + 1],
                        )
                rinv_g = small.tile([P, subtiles], f32, tag="rinv")
                nc.vector.reciprocal(out=rinv_g[:], in_=esum_g[:])
                nc.vector.tensor_mul(
                    og[:],
                    og[:],
                    rinv_g[:, :, None].broadcast_to([P, subtiles, K]),
                )
                nc.sync.dma_start(out=out_v[:, t0:t0 + subtiles, :], in_=og[:])
                t0 += subtiles
    nc.finalize()
    return nc


def get_nc():
    if "nc" not in _CACHE:
        _CACHE["nc"] = _build_bass()
    return _CACHE["nc"]


def _split_hi_lo(a: np.ndarray) -> tuple[np.ndarray, np.ndarray]:
    hi = a.astype(BF16_NP)
    lo = (a - hi.astype(np.float32)).astype(BF16_NP)
    return hi, lo


def prep_inputs(y_pred: np.ndarray, mask: np.ndarray, centers: np.ndarray):
    """Host-side shard prep: valid-timestep slice, per-core transpose,
    bf16 hi/lo splits, contiguous per-DMA packing."""
    x = np.ascontiguousarray(y_pred.reshape(B, T, D))
    masktime = np.asarray(mask).reshape(B, T, D)[0, :, 0]
    valid_idx = np.nonzero(masktime == 0)[0][:VALID_T]
    assert valid_idx.shape[0] == VALID_T
    if valid_idx[0] == 0 and valid_idx[-1] == VALID_T - 1:
        xv = x[:, :VALID_T]                    # [B, VALID_T, D]
    else:
        xv = x[:, valid_idx]

    centers = np.asarray(centers, dtype=np.float32)
    cth, ctl = _split_hi_lo((2.0 * centers).T)              # [D, K] each
    # [h, c, p, k] -> [h, p, c, k] contiguous
    ct_blocks = [
        np.ascontiguousarray(h.reshape(D_CHUNKS, P, K).transpose(1, 0, 2)).ravel()
        for h in (cth, ctl)
    ]
    ctp = np.ascontiguousarray(np.concatenate(ct_blocks))

    negc2 = -(centers.astype(np.float64) ** 2).sum(axis=1)  # [K]
    b1 = negc2.astype(BF16_NP)
    r1 = negc2 - b1.astype(np.float64)
    b2 = r1.astype(BF16_NP)
    b3 = (r1 - b2.astype(np.float64)).astype(BF16_NP)
    bias3 = np.ascontiguousarray(np.stack([b1, b2, b3]))    # [3, K]
    ones3 = np.ones((3, P), dtype=BF16_NP)

    in_maps = []
    for core in range(N_CORES):
        xc = xv[core * B_PER_CORE:(core + 1) * B_PER_CORE].reshape(ROWS, D)
        xTc = np.ascontiguousarray(xc.T)                    # [D, ROWS]
        xh, xl = _split_hi_lo(xTc)
        # [h, c, p, row] -> [p, h, c, row]
        base = np.stack([xh, xl]).reshape(2, D_CHUNKS, P, ROWS).transpose(2, 0, 1, 3)
        blocks = []
        r0 = 0
        for R in GROUPS:
            blocks.append(np.ascontiguousarray(base[:, :, :, r0:r0 + R]).ravel())
            r0 += R
        xp = np.concatenate(blocks)
        assert xp.shape[0] == X_TOTAL
        in_maps.append({"xp": xp, "ctp": ctp, "bias3": bias3, "ones3": ones3})
    return in_maps


def kernel(y_pred: np.ndarray, mask: np.ndarray, centers: np.ndarray,
           **run_kwargs) -> np.ndarray:
    in_maps = prep_inputs(y_pred, mask, centers)
    nc = get_nc()
    res = run_bass_kernel_spmd(nc, in_maps, core_ids=list(range(N_CORES)),
                               **run_kwargs)
    _CACHE["last_results"] = res
    out = np.concatenate(
        [r["out"].reshape(B_PER_CORE, VALID_T, K) for r in res.results], axis=0
    )
    return out.astype(np.float32, copy=False)


# revision 37
# speedup vs baseline: 1.0959x; 1.0959x over previous
"""Batch-assign-probability (VQ codebook softmax) kernel for 8 Trainium2 cores.

Math: for each valid row x (D=512), over K=256 centers c_k:
    softmax_k(-||x - c_k||^2) == softmax_k(2 x.c_k - ||c_k||^2)
(the ||x||^2 term is constant over k and cancels in softmax).

Sharding: batch B=16 split across 8 cores (2 batches = 2048 valid rows per
core); the small centers table is replicated. Host prep: slice the valid
(unmasked) timesteps, transpose x to [D, rows] so the contraction dim lands
on SBUF partitions, fold the 2x scale into ct = (2*centers)^T, and split
x / ct into bf16 hi+lo pairs for a 3-pass full-rate matmul:
    x.ct ~= xh.cth + xh.ctl + xl.cth        (error ~2e-4 relative)
The -||c||^2 bias is folded in as one contraction-dim-3 matmul against a
3-level bf16 split of the bias (ones rows on the x side). Host packs each
DMA's source region fully contiguous (8KB-per-partition runs).

Device (per core, Tile framework):
  - load ct hi/lo + bias once; stream x hi/lo in row-groups (small first
    group so the PE starts early, small last group so the exposed softmax
    tail is short); per 128-row tile: 12 bf16 matmuls + 1 bias matmul ->
    PSUM logits [128,256]; reduce_max(negate) -> ACT exp(bias=-max,
    accum sum) -> reciprocal -> scale -> group out DMA.
"""

import numpy as np
import ml_dtypes

import concourse.bacc as bacc
import concourse.tile as tile
from concourse import mybir
from concourse.bass_utils import run_bass_kernel_spmd

B, T, W, C, K = 16, 2048, 512, 1, 256
VALID_T = 1024
D = W * C                       # 512
N_CORES = 8
B_PER_CORE = B // N_CORES       # 2
ROWS = B_PER_CORE * VALID_T     # 2048 rows per core
P = 128
D_CHUNKS = D // P               # 4
GROUPS = [256, 256, 512, 512, 384, 128]   # rows per x/out DMA group
N_WARM_MM = 10                  # dummy matmuls to lift the PE HAM clock-gate
assert sum(GROUPS) == ROWS
X_TOTAL = P * 2 * D_CHUNKS * ROWS    # flat bf16 element count of x param

BF16_NP = ml_dtypes.bfloat16

_CACHE: dict = {}


def _build_bass():
    f32 = mybir.dt.float32
    bf16 = mybir.dt.bfloat16
    nc = bacc.Bacc()
    # x hi/lo, group-major, fully contiguous per group: for each group g
    # (R rows), block [128p, 2h, 4c, R] flattened.
    xp = nc.declare_dram_parameter("xp", [X_TOTAL], bf16, isOutput=False)
    # ct hi block then lo block, each [128p, 4c, 256k] contiguous.
    ctp = nc.declare_dram_parameter("ctp", [2 * P * D_CHUNKS * K], bf16,
                                    isOutput=False)
    bias3 = nc.declare_dram_parameter("bias3", [3, K], bf16, isOutput=False)
    ones3 = nc.declare_dram_parameter("ones3", [3, P], bf16, isOutput=False)
    out = nc.declare_dram_parameter("out", [ROWS, K], f32, isOutput=True)

    out_v = out.rearrange("(t p) k -> p t k", p=P)       # [128, 16, 256]
    ct_half = P * D_CHUNKS * K

    with tile.TileContext(nc) as tc:
        with (
            tc.tile_pool(name="singles", bufs=1) as singles,
            tc.tile_pool(name="xpool", bufs=1) as xpool,
            tc.tile_pool(name="opool", bufs=3) as opool,
            tc.tile_pool(name="small", bufs=8) as small,
            tc.tile_pool(name="psum", bufs=7, space="PSUM") as psum,
            tc.tile_pool(name="psum_warm", bufs=1, space="PSUM") as psum_warm,
        ):
            # DMA order on the sync HWDGE FIFO: ct_hi -> x group 0 -> ct_lo
            # -> remaining x groups. The first four matmuls only need ct_hi
            # + x0, so the PE starts as early as possible. bias/ones ride
            # the scalar HWDGE ring to stay off the sync FIFO.
            ct_sb = singles.tile([P, 2, D_CHUNKS, K], bf16)
            xgs = []
            xoff = 0

            def x_dma(g, R):
                xg = xpool.tile([P, 2, D_CHUNKS, R], bf16, tag=f"xg{g}")
                n = P * 2 * D_CHUNKS * R
                nc.sync.dma_start(
                    out=xg[:],
                    in_=xp[xoff:xoff + n].rearrange(
                        "(p h c r) -> p h c r", p=P, h=2, c=D_CHUNKS),
                )
                xgs.append(xg)
                return n

            def ct_dma(h):
                nc.sync.dma_start(
                    out=ct_sb[:, h],
                    in_=ctp[h * ct_half:(h + 1) * ct_half].rearrange(
                        "(p c k) -> p c k", p=P, c=D_CHUNKS),
                )

            ct_dma(0)
            xoff += x_dma(0, GROUPS[0])
            ct_dma(1)
            bias_sb = singles.tile([3, K], bf16)
            nc.scalar.dma_start(out=bias_sb[:], in_=bias3[:])
            ones_sb = singles.tile([3, P], bf16)
            nc.scalar.dma_start(out=ones_sb[:], in_=ones3[:])
            for g, R in enumerate(GROUPS[1:], start=1):
                xoff += x_dma(g, R)

            # PE warm-up: dummy matmuls on scratch data keep the PE busy
            # through the HAM activity window while the first x DMA lands,
            # so the real matmul stream runs at 2.4 GHz from the start.
            warm_sb = singles.tile([P, 512], bf16)
            nc.vector.memset(warm_sb[:], 0.0)
            warm_ps = psum_warm.tile([P, 512], f32, tag="warm")
            for _ in range(N_WARM_MM):
                nc.tensor.matmul(
                    warm_ps[:], lhsT=warm_sb[:, :P], rhs=warm_sb[:],
                    start=True, stop=True,
                )

            t0 = 0  # running 128-row tile index
            for g, R in enumerate(GROUPS):
                xg = xgs[g]
                subtiles = R // P
                og = opool.tile([P, subtiles, K], f32, tag="og")
                esum_g = small.tile([P, subtiles], f32, tag="esum")
                # pair subtiles into one full PSUM bank: shared reduce_max,
                # per-group reciprocal + broadcast multiply
                for s0 in range(0, subtiles, 2):
                    pair = min(2, subtiles - s0)
                    ps = psum.tile([P, pair, K], f32, tag="ps")
                    for j in range(pair):
                        s = s0 + j
                        rsl = slice(s * P, (s + 1) * P)
                        first = True
                        for xh_i, ct_i in ((0, 0), (1, 0), (0, 1)):
                            for c in range(D_CHUNKS):
                                nc.tensor.matmul(
                                    ps[:, j, :],
                                    lhsT=xg[:, xh_i, c, rsl],
                                    rhs=ct_sb[:, ct_i, c, :],
                                    start=first,
                                    stop=False,
                                )
                                first = False
                        nc.tensor.matmul(
                            ps[:, j, :],
                            lhsT=ones_sb[:, :],
                            rhs=bias_sb[:, :],
                            start=False,
                            stop=True,
                        )
                    negm = small.tile([P, pair], f32, tag="negm")
                    nc.vector.reduce_max(
                        out=negm[:], in_=ps[:], axis=mybir.AxisListType.X, negate=True
                    )
                    for j in range(pair):
                        nc.scalar.activation(
                            out=og[:, s0 + j, :],
                            in_=ps[:, j, :],
                            func=mybir.ActivationFunctionType.Exp,
                            bias=negm[:, j:j + 1],
                            scale=1.0,
                            accum_out=esum_g[:, s0 + j:s0 + j + 1],
                        )
                rinv_g = small.tile([P, subtiles], f32, tag="rinv")
                nc.vector.reciprocal(out=rinv_g[:], in_=esum_g[:])
                nc.vector.tensor_mul(
                    og[:],
                    og[:],
                    rinv_g[:, :, None].broadcast_to([P, subtiles, K]),
                )
                nc.sync.dma_start(out=out_v[:, t0:t0 + subtiles, :], in_=og[:])
                t0 += subtiles
    nc.finalize()
    return nc


def get_nc():
    if "nc" not in _CACHE:
        _CACHE["nc"] = _build_bass()
    return _CACHE["nc"]


def _split_hi_lo(a: np.ndarray) -> tuple[np.ndarray, np.ndarray]:
    hi = a.astype(BF16_NP)
    lo = (a - hi.astype(np.float32)).astype(BF16_NP)
    return hi, lo


def prep_inputs(y_pred: np.ndarray, mask: np.ndarray, centers: np.ndarray):
    """Host-side shard prep: valid-timestep slice, per-core transpose,
    bf16 hi/lo splits, contiguous per-DMA packing."""
    x = np.ascontiguousarray(y_pred.reshape(B, T, D))
    masktime = np.asarray(mask).reshape(B, T, D)[0, :, 0]
    valid_idx = np.nonzero(masktime == 0)[0][:VALID_T]
    assert valid_idx.shape[0] == VALID_T
    if valid_idx[0] == 0 and valid_idx[-1] == VALID_T - 1:
        xv = x[:, :VALID_T]                    # [B, VALID_T, D]
    else:
        xv = x[:, valid_idx]

    centers = np.asarray(centers, dtype=np.float32)
    cth, ctl = _split_hi_lo((2.0 * centers).T)              # [D, K] each
    # [h, c, p, k] -> [h, p, c, k] contiguous
    ct_blocks = [
        np.ascontiguousarray(h.reshape(D_CHUNKS, P, K).transpose(1, 0, 2)).ravel()
        for h in (cth, ctl)
    ]
    ctp = np.ascontiguousarray(np.concatenate(ct_blocks))

    negc2 = -(centers.astype(np.float64) ** 2).sum(axis=1)  # [K]
    b1 = negc2.astype(BF16_NP)
    r1 = negc2 - b1.astype(np.float64)
    b2 = r1.astype(BF16_NP)
    b3 = (r1 - b2.astype(np.float64)).astype(BF16_NP)
    bias3 = np.ascontiguousarray(np.stack([b1, b2, b3]))    # [3, K]
    ones3 = np.ones((3, P), dtype=BF16_NP)

    in_maps = []
    for core in range(N_CORES):
        xc = xv[core * B_PER_CORE:(core + 1) * B_PER_CORE].reshape(ROWS, D)
        xTc = np.ascontiguousarray(xc.T)                    # [D, ROWS]
        xh, xl = _split_hi_lo(xTc)
        # [h, c, p, row] -> [p, h, c, row]
        base = np.stack([xh, xl]).reshape(2, D_CHUNKS, P, ROWS).transpose(2, 0, 1, 3)
        blocks = []
        r0 = 0
        for R in GROUPS:
            blocks.append(np.ascontiguousarray(base[:, :, :, r0:r0 + R]).ravel())
            r0 += R
        xp = np.concatenate(blocks)
        assert xp.shape[0] == X_TOTAL
        in_maps.append({"xp": xp, "ctp": ctp, "bias3": bias3, "ones3": ones3})
    return in_maps


def kernel(y_pred: np.ndarray, mask: np.ndarray, centers: np.ndarray,
           **run_kwargs) -> np.ndarray:
    in_maps = prep_inputs(y_pred, mask, centers)
    nc = get_nc()
    last_err = None
    for _attempt in range(3):
        try:
            res = run_bass_kernel_spmd(nc, in_maps, core_ids=list(range(N_CORES)),
                                       **run_kwargs)
            break
        except Exception as e:  # transient NRT device errors — retry
            last_err = e
    else:
        raise last_err
    _CACHE["last_results"] = res
    out = np.concatenate(
        [r["out"].reshape(B_PER_CORE, VALID_T, K) for r in res.results], axis=0
    )
    return out.astype(np.float32, copy=False)


# revision 41
# speedup vs baseline: 1.1121x; 1.0148x over previous
"""Batch-assign-probability (VQ codebook softmax) kernel for 8 Trainium2 cores.

Math: for each valid row x (D=512), over K=256 centers c_k:
    softmax_k(-||x - c_k||^2) == softmax_k(2 x.c_k - ||c_k||^2)
(the ||x||^2 term is constant over k and cancels in softmax).

Sharding: batch B=16 split across 8 cores (2 batches = 2048 valid rows per
core); the small centers table is replicated. Host prep: slice the valid
(unmasked) timesteps, transpose x to [D, rows] so the contraction dim lands
on SBUF partitions, fold the 2x scale into ct = (2*centers)^T, and split
x / ct into bf16 hi+lo pairs for a 3-pass full-rate matmul:
    x.ct ~= xh.cth + xh.ctl + xl.cth        (error ~2e-4 relative)
The -||c||^2 bias is folded in as one contraction-dim-3 matmul against a
3-level bf16 split of the bias (ones rows on the x side). Host packs each
DMA's source region fully contiguous (8KB-per-partition runs).

Device (per core, Tile framework):
  - load ct hi/lo + bias once; stream x hi/lo in row-groups (small first
    group so the PE starts early, small last group so the exposed softmax
    tail is short); per 128-row tile: 12 bf16 matmuls + 1 bias matmul ->
    PSUM logits [128,256]; reduce_max(negate) -> ACT exp(bias=-max,
    accum sum) -> reciprocal -> scale -> group out DMA.
"""

import numpy as np
import ml_dtypes

import concourse.bacc as bacc
import concourse.tile as tile
from concourse import mybir
from concourse.bass_utils import run_bass_kernel_spmd

B, T, W, C, K = 16, 2048, 512, 1, 256
VALID_T = 1024
D = W * C                       # 512
N_CORES = 8
B_PER_CORE = B // N_CORES       # 2
ROWS = B_PER_CORE * VALID_T     # 2048 rows per core
P = 128
D_CHUNKS = D // P               # 4
GROUPS = [128, 256, 512, 512, 384, 128, 128]   # rows per x/out DMA group
N_WARM_MM = 10                  # dummy matmuls to lift the PE HAM clock-gate
assert sum(GROUPS) == ROWS
X_TOTAL = P * 2 * D_CHUNKS * ROWS    # flat bf16 element count of x param

BF16_NP = ml_dtypes.bfloat16

_CACHE: dict = {}


def _build_bass():
    f32 = mybir.dt.float32
    bf16 = mybir.dt.bfloat16
    nc = bacc.Bacc()
    # x hi/lo, group-major, fully contiguous per group: for each group g
    # (R rows), block [128p, 2h, 4c, R] flattened.
    xp = nc.declare_dram_parameter("xp", [X_TOTAL], bf16, isOutput=False)
    # ct hi block then lo block, each [128p, 4c, 256k] contiguous.
    ctp = nc.declare_dram_parameter("ctp", [2 * P * D_CHUNKS * K], bf16,
                                    isOutput=False)
    bias3 = nc.declare_dram_parameter("bias3", [3, K], bf16, isOutput=False)
    ones3 = nc.declare_dram_parameter("ones3", [3, P], bf16, isOutput=False)
    out = nc.declare_dram_parameter("out", [ROWS, K], f32, isOutput=True)

    out_v = out.rearrange("(t p) k -> p t k", p=P)       # [128, 16, 256]
    ct_half = P * D_CHUNKS * K

    with tile.TileContext(nc) as tc:
        with (
            tc.tile_pool(name="singles", bufs=1) as singles,
            tc.tile_pool(name="xpool", bufs=1) as xpool,
            tc.tile_pool(name="opool", bufs=3) as opool,
            tc.tile_pool(name="small", bufs=8) as small,
            tc.tile_pool(name="psum", bufs=7, space="PSUM") as psum,
            tc.tile_pool(name="psum_warm", bufs=1, space="PSUM") as psum_warm,
        ):
            # Two HWDGE rings in parallel: the scalar ring carries the small
            # constant loads (ct hi/lo, bias, ones) while the sync ring
            # carries only the x groups — the first x wire overlaps ct's
            # instead of queueing behind it in the FIFO.
            ct_sb = singles.tile([P, 2, D_CHUNKS, K], bf16)
            xgs = []
            xoff = 0

            def x_dma(g, R):
                xg = xpool.tile([P, 2, D_CHUNKS, R], bf16, tag=f"xg{g}")
                n = P * 2 * D_CHUNKS * R
                nc.sync.dma_start(
                    out=xg[:],
                    in_=xp[xoff:xoff + n].rearrange(
                        "(p h c r) -> p h c r", p=P, h=2, c=D_CHUNKS),
                )
                xgs.append(xg)
                return n

            def ct_dma(h):
                nc.sync.dma_start(
                    out=ct_sb[:, h],
                    in_=ctp[h * ct_half:(h + 1) * ct_half].rearrange(
                        "(p c k) -> p c k", p=P, c=D_CHUNKS),
                )

            ct_dma(0)
            xoff += x_dma(0, GROUPS[0])
            ct_dma(1)
            bias_sb = singles.tile([3, K], bf16)
            nc.scalar.dma_start(out=bias_sb[:], in_=bias3[:])
            ones_sb = singles.tile([3, P], bf16)
            nc.scalar.dma_start(out=ones_sb[:], in_=ones3[:])
            for g, R in enumerate(GROUPS[1:], start=1):
                xoff += x_dma(g, R)

            # PE warm-up: dummy matmuls on scratch data keep the PE busy
            # through the HAM activity window while the first x DMA lands,
            # so the real matmul stream runs at 2.4 GHz from the start.
            warm_sb = singles.tile([P, 512], bf16)
            nc.vector.memset(warm_sb[:], 0.0)
            warm_ps = psum_warm.tile([P, 512], f32, tag="warm")
            for _ in range(N_WARM_MM):
                nc.tensor.matmul(
                    warm_ps[:], lhsT=warm_sb[:, :P], rhs=warm_sb[:],
                    start=True, stop=True,
                )

            t0 = 0  # running 128-row tile index
            for g, R in enumerate(GROUPS):
                xg = xgs[g]
                subtiles = R // P
                og = opool.tile([P, subtiles, K], f32, tag="og")
                esum_g = small.tile([P, subtiles], f32, tag="esum")
                # pair subtiles into one full PSUM bank: shared reduce_max,
                # per-group reciprocal + broadcast multiply
                for s0 in range(0, subtiles, 2):
                    pair = min(2, subtiles - s0)
                    ps = psum.tile([P, pair, K], f32, tag="ps")
                    for j in range(pair):
                        s = s0 + j
                        rsl = slice(s * P, (s + 1) * P)
                        first = True
                        for xh_i, ct_i in ((0, 0), (1, 0), (0, 1)):
                            for c in range(D_CHUNKS):
                                nc.tensor.matmul(
                                    ps[:, j, :],
                                    lhsT=xg[:, xh_i, c, rsl],
                                    rhs=ct_sb[:, ct_i, c, :],
                                    start=first,
                                    stop=False,
                                )
                                first = False
                        nc.tensor.matmul(
                            ps[:, j, :],
                            lhsT=ones_sb[:, :],
                            rhs=bias_sb[:, :],
                            start=False,
                            stop=True,
                        )
                    negm = small.tile([P, pair], f32, tag="negm")
                    nc.vector.reduce_max(
                        out=negm[:], in_=ps[:], axis=mybir.AxisListType.X, negate=True
                    )
                    for j in range(pair):
                        nc.scalar.activation(
                            out=og[:, s0 + j, :],
                            in_=ps[:, j, :],
                            func=mybir.ActivationFunctionType.Exp,
                            bias=negm[:, j:j + 1],
                            scale=1.0,
                            accum_out=esum_g[:, s0 + j:s0 + j + 1],
                        )
                rinv_g = small.tile([P, subtiles], f32, tag="rinv")
                nc.vector.reciprocal(out=rinv_g[:], in_=esum_g[:])
                nc.vector.tensor_mul(
                    og[:],
                    og[:],
                    rinv_g[:, :, None].broadcast_to([P, subtiles, K]),
                )
                nc.sync.dma_start(out=out_v[:, t0:t0 + subtiles, :], in_=og[:])
                t0 += subtiles
    nc.finalize()
    return nc


def get_nc():
    if "nc" not in _CACHE:
        _CACHE["nc"] = _build_bass()
    return _CACHE["nc"]


def _split_hi_lo(a: np.ndarray) -> tuple[np.ndarray, np.ndarray]:
    hi = a.astype(BF16_NP)
    lo = (a - hi.astype(np.float32)).astype(BF16_NP)
    return hi, lo


def prep_inputs(y_pred: np.ndarray, mask: np.ndarray, centers: np.ndarray):
    """Host-side shard prep: valid-timestep slice, per-core transpose,
    bf16 hi/lo splits, contiguous per-DMA packing."""
    x = np.ascontiguousarray(y_pred.reshape(B, T, D))
    masktime = np.asarray(mask).reshape(B, T, D)[0, :, 0]
    valid_idx = np.nonzero(masktime == 0)[0][:VALID_T]
    assert valid_idx.shape[0] == VALID_T
    if valid_idx[0] == 0 and valid_idx[-1] == VALID_T - 1:
        xv = x[:, :VALID_T]                    # [B, VALID_T, D]
    else:
        xv = x[:, valid_idx]

    centers = np.asarray(centers, dtype=np.float32)
    cth, ctl = _split_hi_lo((2.0 * centers).T)              # [D, K] each
    # [h, c, p, k] -> [h, p, c, k] contiguous
    ct_blocks = [
        np.ascontiguousarray(h.reshape(D_CHUNKS, P, K).transpose(1, 0, 2)).ravel()
        for h in (cth, ctl)
    ]
    ctp = np.ascontiguousarray(np.concatenate(ct_blocks))

    negc2 = -(centers.astype(np.float64) ** 2).sum(axis=1)  # [K]
    b1 = negc2.astype(BF16_NP)
    r1 = negc2 - b1.astype(np.float64)
    b2 = r1.astype(BF16_NP)
    b3 = (r1 - b2.astype(np.float64)).astype(BF16_NP)
    bias3 = np.ascontiguousarray(np.stack([b1, b2, b3]))    # [3, K]
    ones3 = np.ones((3, P), dtype=BF16_NP)

    in_maps = []
    for core in range(N_CORES):
        xc = xv[core * B_PER_CORE:(core + 1) * B_PER_CORE].reshape(ROWS, D)
        xTc = np.ascontiguousarray(xc.T)                    # [D, ROWS]
        xh, xl = _split_hi_lo(xTc)
        # [h, c, p, row] -> [p, h, c, row]
        base = np.stack([xh, xl]).reshape(2, D_CHUNKS, P, ROWS).transpose(2, 0, 1, 3)
        blocks = []
        r0 = 0
        for R in GROUPS:
            blocks.append(np.ascontiguousarray(base[:, :, :, r0:r0 + R]).ravel())
            r0 += R
        xp = np.concatenate(blocks)
        assert xp.shape[0] == X_TOTAL
        in_maps.append({"xp": xp, "ctp": ctp, "bias3": bias3, "ones3": ones3})
    return in_maps


def kernel(y_pred: np.ndarray, mask: np.ndarray, centers: np.ndarray,
           **run_kwargs) -> np.ndarray:
    in_maps = prep_inputs(y_pred, mask, centers)
    nc = get_nc()
    last_err = None
    for _attempt in range(3):
        try:
            res = run_bass_kernel_spmd(nc, in_maps, core_ids=list(range(N_CORES)),
                                       **run_kwargs)
            break
        except Exception as e:  # transient NRT device errors — retry
            last_err = e
    else:
        raise last_err
    _CACHE["last_results"] = res
    out = np.concatenate(
        [r["out"].reshape(B_PER_CORE, VALID_T, K) for r in res.results], axis=0
    )
    return out.astype(np.float32, copy=False)


# revision 49
# speedup vs baseline: 1.1821x; 1.0629x over previous
"""Batch-assign-probability (VQ codebook softmax) kernel for 8 Trainium2 cores.

Math: for each valid row x (D=512), over K=256 centers c_k:
    softmax_k(-||x - c_k||^2) == softmax_k(2 x.c_k - ||c_k||^2)
(the ||x||^2 term is constant over k and cancels in softmax).

Sharding: batch B=16 split across 8 cores (2 batches = 2048 valid rows per
core); the small centers table is replicated. Host prep: slice the valid
(unmasked) timesteps, transpose x to [D, rows] so the contraction dim lands
on SBUF partitions, fold the 2x scale into ct = (2*centers)^T, and split
x / ct into bf16 hi+lo pairs for a 3-pass full-rate matmul:
    x.ct ~= xh.cth + xh.ctl + xl.cth        (error ~2e-4 relative)
The -||c||^2 bias is folded in as one contraction-dim-3 matmul against a
3-level bf16 split of the bias (ones rows on the x side). Host packs each
DMA's source region fully contiguous (8KB-per-partition runs).

Device (per core, Tile framework):
  - load ct hi/lo + bias once; stream x hi/lo in row-groups (small first
    group so the PE starts early, small last group so the exposed softmax
    tail is short); per 128-row tile: 12 bf16 matmuls + 1 bias matmul ->
    PSUM logits [128,256]; reduce_max(negate) -> ACT exp(bias=-max,
    accum sum) -> reciprocal -> scale -> group out DMA.
"""

import numpy as np
import ml_dtypes

import concourse.bacc as bacc
import concourse.tile as tile
from concourse import mybir
from concourse.bass_utils import run_bass_kernel_spmd

B, T, W, C, K = 16, 2048, 512, 1, 256
VALID_T = 1024
D = W * C                       # 512
N_CORES = 8
B_PER_CORE = B // N_CORES       # 2
ROWS = B_PER_CORE * VALID_T     # 2048 rows per core
P = 128
D_CHUNKS = D // P               # 4
GROUPS = [128, 256, 512, 512, 384, 128, 128]   # rows per x/out DMA group
N_WARM_MM = 8                  # dummy matmuls to lift the PE HAM clock-gate
assert sum(GROUPS) == ROWS
X_TOTAL = P * 2 * D_CHUNKS * ROWS    # flat bf16 element count of x param

BF16_NP = ml_dtypes.bfloat16

_CACHE: dict = {}


def _build_bass():
    f32 = mybir.dt.float32
    bf16 = mybir.dt.bfloat16
    nc = bacc.Bacc()
    # x hi/lo, group-major, fully contiguous per group: for each group g
    # (R rows), block [128p, 2h, 4c, R] flattened.
    xp = nc.declare_dram_parameter("xp", [X_TOTAL], bf16, isOutput=False)
    # ct hi block then lo block, each [128p, 4c, 256k] contiguous.
    ctp = nc.declare_dram_parameter("ctp", [2 * P * D_CHUNKS * K], bf16,
                                    isOutput=False)
    bias3 = nc.declare_dram_parameter("bias3", [P, K], bf16, isOutput=False)
    ones3 = nc.declare_dram_parameter("ones3", [P, P], bf16, isOutput=False)
    out = nc.declare_dram_parameter("out", [ROWS, K], f32, isOutput=True)

    out_v = out.rearrange("(t p) k -> p t k", p=P)       # [128, 16, 256]
    ct_half = P * D_CHUNKS * K

    with tile.TileContext(nc) as tc:
        with (
            tc.tile_pool(name="singles", bufs=1) as singles,
            tc.tile_pool(name="xpool", bufs=1) as xpool,
            tc.tile_pool(name="opool", bufs=3) as opool,
            tc.tile_pool(name="small", bufs=8) as small,
            tc.tile_pool(name="psum", bufs=7, space="PSUM") as psum,
            tc.tile_pool(name="psum_warm", bufs=1, space="PSUM") as psum_warm,
        ):
            # Two HWDGE rings in parallel: the scalar ring carries the small
            # constant loads (ct hi/lo, bias, ones) while the sync ring
            # carries only the x groups — the first x wire overlaps ct's
            # instead of queueing behind it in the FIFO.
            ct_sb = singles.tile([P, 2, D_CHUNKS, K], bf16)
            xgs = []
            xoff = 0

            def x_dma(g, R, split=False):
                xg = xpool.tile([P, 2, D_CHUNKS, R], bf16, tag=f"xg{g}")
                n = P * 2 * D_CHUNKS * R
                src = xp[xoff:xoff + n].rearrange(
                    "(p h c r) -> p h c r", p=P, h=2, c=D_CHUNKS)
                if split:
                    # hi half on the sync ring (gates the first matmuls),
                    # lo half in parallel on the scalar ring
                    nc.sync.dma_start(out=xg[:, 0], in_=src[:, 0])
                    nc.scalar.dma_start(out=xg[:, 1], in_=src[:, 1])
                else:
                    nc.sync.dma_start(out=xg[:], in_=src)
                xgs.append(xg)
                return n

            def ct_dma(h, eng=None):
                (eng or nc.sync).dma_start(
                    out=ct_sb[:, h],
                    in_=ctp[h * ct_half:(h + 1) * ct_half].rearrange(
                        "(p c k) -> p c k", p=P, c=D_CHUNKS),
                )

            ct_dma(0)
            xoff += x_dma(0, GROUPS[0], split=True)
            ct_dma(1)
            bias_sb = singles.tile([P, K], bf16)
            nc.scalar.dma_start(out=bias_sb[:], in_=bias3[:])
            ones_sb = singles.tile([P, P], bf16)
            nc.scalar.dma_start(out=ones_sb[:], in_=ones3[:])
            for g, R in enumerate(GROUPS[1:], start=1):
                xoff += x_dma(g, R)

            # PE warm-up: dummy matmuls on scratch data keep the PE busy
            # through the HAM activity window while the first x DMA lands,
            # so the real matmul stream runs at 2.4 GHz from the start.
            warm_sb = singles.tile([P, 512], bf16)
            nc.gpsimd.memset(warm_sb[:], 0.0)
            warm_ps = psum_warm.tile([P, 512], f32, tag="warm")
            for _ in range(N_WARM_MM):
                nc.tensor.matmul(
                    warm_ps[:], lhsT=warm_sb[:, :P], rhs=warm_sb[:],
                    start=True, stop=True,
                )

            t0 = 0  # running 128-row tile index
            for g, R in enumerate(GROUPS):
                xg = xgs[g]
                subtiles = R // P
                og = opool.tile([P, subtiles, K], f32, tag="og")
                esum_g = small.tile([P, subtiles], f32, tag="esum")
                # pair subtiles into one full PSUM bank: shared reduce_max,
                # per-group reciprocal + broadcast multiply
                for s0 in range(0, subtiles, 2):
                    pair = min(2, subtiles - s0)
                    ps = psum.tile([P, pair, K], f32, tag="ps")
                    for j in range(pair):
                        s = s0 + j
                        rsl = slice(s * P, (s + 1) * P)
                        first = True
                        for xh_i, ct_i in ((0, 0), (1, 0), (0, 1)):
                            for c in range(D_CHUNKS):
                                nc.tensor.matmul(
                                    ps[:, j, :],
                                    lhsT=xg[:, xh_i, c, rsl],
                                    rhs=ct_sb[:, ct_i, c, :],
                                    start=first,
                                    stop=False,
                                )
                                first = False
                        nc.tensor.matmul(
                            ps[:, j, :],
                            lhsT=ones_sb[:, :],
                            rhs=bias_sb[:, :],
                            start=False,
                            stop=True,
                        )
                    negm = small.tile([P, pair], f32, tag="negm")
                    nc.vector.reduce_max(
                        out=negm[:], in_=ps[:], axis=mybir.AxisListType.X, negate=True
                    )
                    for j in range(pair):
                        nc.scalar.activation(
                            out=og[:, s0 + j, :],
                            in_=ps[:, j, :],
                            func=mybir.ActivationFunctionType.Exp,
                            bias=negm[:, j:j + 1],
                            scale=1.0,
                            accum_out=esum_g[:, s0 + j:s0 + j + 1],
                        )
                rinv_g = small.tile([P, subtiles], f32, tag="rinv")
                nc.vector.reciprocal(out=rinv_g[:], in_=esum_g[:])
                nc.vector.tensor_mul(
                    og[:],
                    og[:],
                    rinv_g[:, :, None].broadcast_to([P, subtiles, K]),
                )
                nc.sync.dma_start(out=out_v[:, t0:t0 + subtiles, :], in_=og[:])
                t0 += subtiles
    nc.finalize()
    return nc


def get_nc():
    if "nc" not in _CACHE:
        _CACHE["nc"] = _build_bass()
    return _CACHE["nc"]


def _split_hi_lo(a: np.ndarray) -> tuple[np.ndarray, np.ndarray]:
    hi = a.astype(BF16_NP)
    lo = (a - hi.astype(np.float32)).astype(BF16_NP)
    return hi, lo


def prep_inputs(y_pred: np.ndarray, mask: np.ndarray, centers: np.ndarray):
    """Host-side shard prep: valid-timestep slice, per-core transpose,
    bf16 hi/lo splits, contiguous per-DMA packing."""
    x = np.ascontiguousarray(y_pred.reshape(B, T, D))
    masktime = np.asarray(mask).reshape(B, T, D)[0, :, 0]
    valid_idx = np.nonzero(masktime == 0)[0][:VALID_T]
    assert valid_idx.shape[0] == VALID_T
    if valid_idx[0] == 0 and valid_idx[-1] == VALID_T - 1:
        xv = x[:, :VALID_T]                    # [B, VALID_T, D]
    else:
        xv = x[:, valid_idx]

    centers = np.asarray(centers, dtype=np.float32)
    cth, ctl = _split_hi_lo((2.0 * centers).T)              # [D, K] each
    # [h, c, p, k] -> [h, p, c, k] contiguous
    ct_blocks = [
        np.ascontiguousarray(h.reshape(D_CHUNKS, P, K).transpose(1, 0, 2)).ravel()
        for h in (cth, ctl)
    ]
    ctp = np.ascontiguousarray(np.concatenate(ct_blocks))

    negc2 = -(centers.astype(np.float64) ** 2).sum(axis=1)  # [K]
    b1 = negc2.astype(BF16_NP)
    r1 = negc2 - b1.astype(np.float64)
    b2 = r1.astype(BF16_NP)
    b3 = (r1 - b2.astype(np.float64)).astype(BF16_NP)
    # contraction padded to 128 rows so the bias matmul's weight load
    # matches the regular [128,128] shape (keeps PE weight pipelining)
    bias3 = np.zeros((P, K), dtype=BF16_NP)
    bias3[0], bias3[1], bias3[2] = b1, b2, b3
    ones3 = np.ones((P, P), dtype=BF16_NP)

    in_maps = []
    for core in range(N_CORES):
        xc = xv[core * B_PER_CORE:(core + 1) * B_PER_CORE].reshape(ROWS, D)
        xTc = np.ascontiguousarray(xc.T)                    # [D, ROWS]
        xh, xl = _split_hi_lo(xTc)
        # [h, c, p, row] -> [p, h, c, row]
        base = np.stack([xh, xl]).reshape(2, D_CHUNKS, P, ROWS).transpose(2, 0, 1, 3)
        blocks = []
        r0 = 0
        for R in GROUPS:
            blocks.append(np.ascontiguousarray(base[:, :, :, r0:r0 + R]).ravel())
            r0 += R
        xp = np.concatenate(blocks)
        assert xp.shape[0] == X_TOTAL
        in_maps.append({"xp": xp, "ctp": ctp, "bias3": bias3, "ones3": ones3})
    return in_maps


def kernel(y_pred: np.ndarray, mask: np.ndarray, centers: np.ndarray,
           **run_kwargs) -> np.ndarray:
    in_maps = prep_inputs(y_pred, mask, centers)
    nc = get_nc()
    last_err = None
    for _attempt in range(3):
        try:
            res = run_bass_kernel_spmd(nc, in_maps, core_ids=list(range(N_CORES)),
                                       **run_kwargs)
            break
        except Exception as e:  # transient NRT device errors — retry
            last_err = e
    else:
        raise last_err
    _CACHE["last_results"] = res
    out = np.concatenate(
        [r["out"].reshape(B_PER_CORE, VALID_T, K) for r in res.results], axis=0
    )
    return out.astype(np.float32, copy=False)
